# revision 4
# baseline (speedup 1.0000x reference)
"""Trainium2 Bass kernel v2: 2-layer bidirectional AllenNLP LSTM.

B=64, T=512, D_IN=512, H=500. Data-parallel over batch: 8 seqs/core x 8 cores.

Design (per core):
- Input projection per layer: x_gates = x @ W_ih'.T; layer-0 stationary x^T
  comes pre-transposed from the host (x0T), layer-1 stationary comes directly
  from the H-major y0 intermediate, so no on-device transposes in inproj.
  Gate columns host-permuted to [i, f, o, 2*g]. All masking is folded into
  x_gates here: i/o gates get -50*(1-m), f gets +50*(1-m), so the recurrence
  needs zero mask ops and h == y exactly (validated vs reference).
- Recurrence: per direction, gates land in one PSUM tile [128, 500] with one
  gate per 32-partition column group (i@0, f@32, o@64, 2g@96): 16 matmuls
  (4 col groups x 4 K-chunks) with h^T stationary and W_hh streamed; the 4
  col groups run concurrently on the PE array. x_gates(t)+bias are injected
  via 8 identity rows + 1 ones row in K-chunk 0 (chunks 119+9/127/127/127).
- One sigmoid activation covers all gates (tanh(z) = 2*sig(2z)-1, g-gate
  pre-scaled by 2). The activated gate tile [128, 500] is PE-transposed in 4
  chunks into an H-major PSUM tile [127, 4x128]; the whole c/h elementwise
  chain then runs on dense [127, 4, 8] APs (~32 elem/lane) and the h op
  writes the next step's h^T stationary tile directly (bf16). No h transpose,
  no mask ops, no per-step DMA except the 2 x_gates injections.
- y output (== h) is staged H-major and DMA'd to an H-major DRAM layout;
  the host un-permutes. Layer-0 y feeds layer-1 inproj as its stationary.
"""

import os
import sys
from contextlib import ExitStack

import numpy as np

sys.path.insert(0, "/opt/trn_rl_repo")

import concourse.bass as bass
import concourse.bacc as bacc
import concourse.mybir as mybir
import concourse.tile as tile
from concourse.bass_utils import run_bass_kernel_spmd

B, T, D_IN, H, G = 64, 512, 512, 500, 2000  # G = 4*H
NCORES = 8
BS = B // NCORES  # 8 seqs per core
F32 = mybir.dt.float32
F32R = mybir.dt.float32r
BF16 = mybir.dt.bfloat16
ds = bass.ds
ts = bass.ts
PE = mybir.EngineType.PE
DVE = mybir.EngineType.DVE
ACT = mybir.EngineType.Activation
SIG = mybir.ActivationFunctionType.Sigmoid
TANH = mybir.ActivationFunctionType.Tanh
MUL = mybir.AluOpType.mult
ADD = mybir.AluOpType.add
SUB = mybir.AluOpType.subtract

# K chunks of the recurrence contraction: chunk 0 = h[0:119] + 8 identity
# rows (x_gates inject) + 1 ones row (bias inject) = 128.
RCH = [(0, 119), (119, 127), (246, 127), (373, 127)]
MOFF = 50.0  # mask offset magnitude on i/f/o pre-activations

UNROLL = 16
HB = 8  # steps per staging half-block

_env_t = os.environ.get("LSTM_T")
TT = int(_env_t) if _env_t else T  # reduced T for smoke tests


def _build_nc(t_steps: int):
    nt = t_steps // 16
    nc = bacc.Bacc("TRN2", target_bir_lowering=False, debug=False,
                   num_devices=NCORES)

    x0T = nc.dram_tensor("x0T", [4, 128, t_steps * BS], F32R,
                         kind="ExternalInput").ap()
    wih0 = nc.dram_tensor("wih0", [2, D_IN, G], F32R, kind="ExternalInput").ap()
    wih1 = nc.dram_tensor("wih1", [2, 2 * H, G], BF16, kind="ExternalInput").ap()
    whh = nc.dram_tensor("whh", [2, 2, H, G], BF16, kind="ExternalInput").ap()
    bias = nc.dram_tensor("bias", [2, 2, 1, G], BF16, kind="ExternalInput").ap()
    xgid = nc.dram_tensor("xgid", [9, 16], BF16, kind="ExternalInput").ap()
    idbf = nc.dram_tensor("idbf", [128, 128], BF16, kind="ExternalInput").ap()
    moff = nc.dram_tensor("moff", [nt, 128, 2], F32, kind="ExternalInput").ap()
    out = nc.dram_tensor("out", [2, 127, 4, t_steps, BS], F32,
                         kind="ExternalOutput").ap()

    xga = nc.dram_tensor("xga", [2, t_steps, BS, G], BF16, kind="Internal").ap()
    xgb = nc.dram_tensor("xgb", [2, t_steps, BS, G], BF16, kind="Internal").ap()
    y0h = nc.dram_tensor("y0h", [2, 127, 4, t_steps, BS], BF16,
                         kind="Internal").ap()

    with tile.TileContext(nc) as tc:
        with tc.tile_pool(name="gconst", bufs=1) as gconst:
            idt = gconst.tile([128, 128], BF16, tag="idt")
            nc.sync.dma_start(idt[:, :], idbf)
            # rec W tiles prefetch during the preceding inproj phase
            with ExitStack() as c0:
                st0 = _rec_prep(nc, tc, c0, 0, whh, bias, xgid)
                _inproj0(nc, tc, x0T, wih0, moff, xga, nt)
                tc.strict_bb_all_engine_barrier()
                _rec(nc, tc, 0, t_steps, xga, st0, idt, y0h, BF16)
            tc.strict_bb_all_engine_barrier()
            with ExitStack() as c1:
                st1 = _rec_prep(nc, tc, c1, 1, whh, bias, xgid)
                _inproj1(nc, tc, y0h, wih1, moff, xgb, nt)
                tc.strict_bb_all_engine_barrier()
                _rec(nc, tc, 1, t_steps, xgb, st1, idt, out, F32)
    nc.compile()
    return nc


def _emit_gates(nc, pool, gp, mofft, xg_out, d, tt):
    """PSUM gate tile [128, 2048] -> bf16 with mask offsets -> DRAM."""
    for n in range(4):
        gs = pool.tile([128, 500], BF16, tag="gs")
        sl = gp[:, 512 * n:512 * n + 500]
        if n == 3:  # 2g: no offset
            nc.vector.tensor_copy(gs[:, :], sl)
        elif n == 1:  # f: +MOFF*(1-m)
            nc.vector.tensor_scalar(gs[:, :], sl, mofft[:, 1:2], None, ADD)
        else:  # i, o: -MOFF*(1-m)
            nc.vector.tensor_scalar(gs[:, :], sl, mofft[:, 0:1], None, ADD)
        nc.sync.dma_start(xg_out[d, ts(tt, 16), :, ts(n, 500)], gs[:, :])


def _inproj0(nc, tc, x0T, wih, moff, xg_out, nt):
    """xg_out[d, t, b, :] = x[t, b, :] @ wih[d] (+ mask offsets)."""
    with ExitStack() as ctx:
        wpool = ctx.enter_context(tc.tile_pool(name="ipw0", bufs=1))
        w_sb = []
        for d in range(2):
            row = []
            for k in range(4):
                t = wpool.tile([128, G], F32R, tag=f"w{d}_{k}")
                nc.sync.dma_start(t[:, :], wih[d, ts(k, 128), :])
                row.append(t)
            w_sb.append(row)
        pool = ctx.enter_context(tc.tile_pool(name="ip0", bufs=3))
        xpool = ctx.enter_context(tc.tile_pool(name="ipx0", bufs=2))
        psum = ctx.enter_context(
            tc.tile_pool(name="ipp0", bufs=2, space="PSUM"))

        for tt in range(nt):
            mofft = pool.tile([128, 2], F32, tag="moff")
            nc.gpsimd.dma_start(mofft[:, :], moff[tt])
            xT = []
            for k in range(4):
                xt = xpool.tile([128, 128], F32R, tag=f"xT{k}")
                nc.gpsimd.dma_start(xt[:, :], x0T[k, :, ts(tt, 128)])
                xT.append(xt)
            for d in range(2):
                gp = psum.tile([128, 2048], F32, tag="gp")
                for n in range(4):
                    for k in range(4):
                        nc.tensor.matmul(
                            gp[:, 512 * n:512 * n + 500], lhsT=xT[k][:, :],
                            rhs=w_sb[d][k][:, ts(n, 500)],
                            start=(k == 0), stop=(k == 3))
                _emit_gates(nc, pool, gp, mofft, xg_out, d, tt)


def _inproj1(nc, tc, y0h, wih, moff, xg_out, nt):
    """xg_out[d, t, b, :] = y0[t, b, :] @ wih[d]; stationary from H-major y0h."""
    with ExitStack() as ctx:
        wpool = ctx.enter_context(tc.tile_pool(name="ipw1", bufs=1))
        w_sb = []
        for d in range(2):
            row = []
            for dd in range(2):
                for k in range(4):
                    off, cnt = RCH[k]
                    t = wpool.tile([cnt, G], BF16, tag=f"w{d}_{dd}{k}")
                    nc.sync.dma_start(t[:, :],
                                      wih[d, ds(500 * dd + off, cnt), :])
                    row.append(t)
            w_sb.append(row)
        pool = ctx.enter_context(tc.tile_pool(name="ip1", bufs=3))
        xpool = ctx.enter_context(tc.tile_pool(name="ipx1", bufs=2))
        psum = ctx.enter_context(
            tc.tile_pool(name="ipp1", bufs=2, space="PSUM"))

        for tt in range(nt):
            mofft = pool.tile([128, 2], F32, tag="moff")
            nc.gpsimd.dma_start(mofft[:, :], moff[tt])
            xT = []
            for dd in range(2):
                for k in range(4):
                    off, cnt = RCH[k]
                    xt = xpool.tile([cnt, 128], BF16, tag=f"xT{dd}{k}")
                    nc.gpsimd.dma_start(xt[:, :], y0h[dd, 0:cnt, k, ts(tt, 16), :])
                    xT.append(xt)
            for d in range(2):
                gp = psum.tile([128, 2048], F32, tag="gp")
                for n in range(4):
                    for c in range(8):
                        cnt = RCH[c % 4][1]
                        nc.tensor.matmul(
                            gp[:, 512 * n:512 * n + 500],
                            lhsT=xT[c][0:cnt, :],
                            rhs=w_sb[d][c][0:cnt, ts(n, 500)],
                            start=(c == 0), stop=(c == 7))
                _emit_gates(nc, pool, gp, mofft, xg_out, d, tt)


def _rec_prep(nc, tc, ctx, layer, whh, bias, xgid):
    """Allocate recurrence tiles and issue W loads (overlaps prior phase)."""
    cpool = ctx.enter_context(tc.tile_pool(name=f"rc{layer}", bufs=1))
    # Streamed W tiles. Chunk 0 in a 4-deep rotation (rows 119:127 get
    # x_gates rows per step via static SBUF->SBUF DMA, row 127 = bias,
    # 3 steps of inject lead); chunks 1-3 static.
    rhs0, rhs_rest = [], []
    for d in range(2):
        quad = []
        for p in range(4):
            t = cpool.tile([128, G], BF16, tag=f"r0_{d}{p}")
            nc.sync.dma_start(t[0:119, :], whh[layer, d, 0:119, :])
            nc.sync.dma_start(t[127:128, :], bias[layer, d, :, :])
            quad.append(t)
        rhs0.append(quad)
        rest = []
        for k in range(1, 4):
            off, cnt = RCH[k]
            t = cpool.tile([cnt, G], BF16, tag=f"r{k}_{d}")
            nc.sync.dma_start(t[:, :], whh[layer, d, ds(off, cnt), :])
            rest.append(t)
        rhs_rest.append(rest)
    # Stationary h^T tiles per direction [128, 32]: col 8k+b; chunk-0
    # rows 119:128 hold the static identity8 + ones block. Separate
    # tiles per direction so the anti-phase pipeline has no false deps.
    sets = []
    for d in range(2):
        pair = []
        for p in range(2):
            t = cpool.tile([128, 32], BF16, tag=f"hT{d}{p}",
                           name=f"hT{d}{p}")
            nc.vector.memset(t[:, :], 0.0)
            nc.sync.dma_start(t[119:128, 0:8], xgid[0:9, 0:8])
            pair.append(t)
        sets.append(pair)
    # c state + dense scratch, all [127, 32] = (4 chunks x 8 batch)
    c_t = []
    for d in range(2):
        t = cpool.tile([127, 32], F32, tag=f"c{d}")
        nc.vector.memset(t[:, :], 0.0)
        c_t.append(t)
    stg = [cpool.tile([128, G], BF16, tag=f"stg{h}", name=f"stg{h}")
           for h in range(2)]
    cm1 = cpool.tile([127, 1], F32, tag="cm1")
    nc.vector.memset(cm1[:, :], -1.0)
    return rhs0, rhs_rest, sets, c_t, stg, cm1


def _rec(nc, tc, layer, t_steps, xg, st, idt, y_out, y_dt):
    """Bidirectional recurrence; y == h streamed out H-major."""
    rhs0, rhs_rest, sets, c_t, stg, cm1 = st
    with ExitStack() as ctx:
        gpool = ctx.enter_context(
            tc.tile_pool(name=f"rg{layer}", bufs=2, space="PSUM"))
        tpool = ctx.enter_context(
            tc.tile_pool(name=f"rt{layer}", bufs=2, space="PSUM"))
        spool = ctx.enter_context(tc.tile_pool(name=f"rs{layer}", bufs=2))
        ypool = ctx.enter_context(tc.tile_pool(name=f"ry{layer}", bufs=2))

        tc.strict_bb_all_engine_barrier()

        def body(iv0, unroll):
            assert unroll == UNROLL
            for h in range(2):
                nc.gpsimd.dma_start(stg[h][0:64, :],
                                    xg[0, ds(iv0 + h * HB, HB), :, :])
                nc.gpsimd.dma_start(
                    stg[h][64:128, :],
                    xg[1, ds(t_steps - HB - iv0 - h * HB, HB), :, :])
            # y staging: [127, (k:4, i:16, b:8)] per direction, full block
            ys = [ypool.tile([127, 4 * UNROLL * BS], y_dt, tag=f"ys{d}",
                             name=f"ys{d}") for d in range(2)]
            def inject(d, j):
                half, jj = divmod(j, HB)
                row = 8 * jj if d == 0 else 64 + 8 * (HB - 1 - jj)
                eng = nc.sync if d == 0 else nc.gpsimd
                eng.dma_start(rhs0[d][j % 4][119:127, :],
                              stg[half][row:row + 8, :])

            def mm(d, j):
                gpd = gpool.tile([128, 500], F32, tag=f"gp{d}", name=f"gp{d}")
                for k in (3, 1, 2, 0):
                    rt = rhs0[d][j % 4] if k == 0 else rhs_rest[d][k - 1]
                    kp = 128 if k == 0 else RCH[k][1]
                    for g in range(4):
                        nc.tensor.matmul(
                            gpd[32 * g:32 * g + 8, :],
                            lhsT=sets[d][j % 2][0:kp, 8 * k:8 * k + 8],
                            rhs=rt[0:kp, ts(g, 500)],
                            start=(k == 3), stop=(k == 0),
                            tile_position=(0, 32 * g))
                return gpd

            def sig(d, gpd):
                ggd = spool.tile([128, 500], BF16, tag=f"gg{d}", name=f"gg{d}")
                nc.scalar.activation(ggd[:, :], gpd[:, :], SIG)
                return ggd

            def tr(d, ggd):
                xt = tpool.tile([127, 512], BF16, tag=f"xt{d}", name=f"xt{d}")
                for k in range(4):
                    off, cnt = RCH[k]
                    nc.tensor.transpose(xt[0:cnt, ts(k, 128)],
                                        ggd[:, ds(off, cnt)],
                                        idt[0:128, 0:128])
                return xt

            def dense(specs):
                # specs: list of (d, j, xt); emit op-position-major so the
                # active chains interleave on the DVE FIFO
                st = []
                for d, j, xt in specs:
                    xr = xt[:, :].rearrange("p (k g b) -> p k g b", k=4, g=4)
                    gsc = spool.tile([127, 32], F32, tag=f"gsc{d}",
                                     name=f"gsc{d}")
                    igt = spool.tile([127, 32], F32, tag=f"ig{d}",
                                     name=f"ig{d}")
                    fct = spool.tile([127, 32], F32, tag=f"fc{d}",
                                     name=f"fc{d}")
                    csd = spool.tile([127, 32], F32, tag=f"cs{d}",
                                     name=f"cs{d}")
                    st.append((d, j, xr,
                               gsc[:, :].rearrange("p (k b) -> p k b", k=4),
                               igt[:, :].rearrange("p (k b) -> p k b", k=4),
                               fct[:, :].rearrange("p (k b) -> p k b", k=4),
                               c_t[d][:, :].rearrange("p (k b) -> p k b", k=4),
                               csd))
                for d, j, xr, gsr, igr, fcr, cr, csd in st:
                    # g' = 2*sig(2z) - 1 on ACT (keeps the DVE chain shorter)
                    nc.scalar.activation(gsr, xr[:, :, 3, 0:8],
                                         mybir.ActivationFunctionType.Identity,
                                         bias=cm1[:, 0:1], scale=2.0)
                for d, j, xr, gsr, igr, fcr, cr, csd in st:
                    nc.vector.tensor_tensor(igr, xr[:, :, 0, 0:8], gsr, MUL)
                for d, j, xr, gsr, igr, fcr, cr, csd in st:
                    nc.vector.tensor_tensor(fcr, xr[:, :, 1, 0:8], cr, MUL)
                for d, j, xr, gsr, igr, fcr, cr, csd in st:
                    nc.vector.tensor_tensor(cr, igr, fcr, ADD)  # c_new
                for d, j, xr, gsr, igr, fcr, cr, csd in st:
                    nc.scalar.activation(csd[:, :], c_t[d][:, :], TANH)
                for d, j, xr, gsr, igr, fcr, cr, csd in st:
                    csr = csd[:, :].rearrange("p (k b) -> p k b", k=4)
                    # h = o * tanh(c), written straight into h^T stationary
                    sr = sets[d][1 - j % 2][:, :].rearrange(
                        "p (k b) -> p k b", k=4)
                    nc.vector.tensor_tensor(sr[0:119], xr[0:119, :, 2, 0:8],
                                            csr[0:119], MUL)
                    # rows 119:127 of chunks 1-3 (32-aligned base; rows
                    # 96:119 are recomputed with identical values)
                    nc.vector.tensor_tensor(sr[96:127, 1:4],
                                            xr[96:127, 1:4, 2, 0:8],
                                            csr[96:127, 1:4], MUL)
                    # y == h: stage H-major (bwd t-reversed within block)
                    yr = ys[d][:, :].rearrange("p (k i b) -> p k i b",
                                               k=4, i=UNROLL)
                    nc.gpsimd.tensor_copy(
                        yr[:, :, j if d == 0 else UNROLL - 1 - j, :],
                        sr[0:127])

            # anti-phase pipeline: bwd lags fwd by one step so each
            # direction's matmuls fill the other's elementwise window
            for i in range(unroll):
                inject(0, i)
                inject(1, i)
                gpf = mm(0, i)
                ggf = sig(0, gpf)
                if i > 0:
                    gpb = mm(1, i - 1)
                xtf = tr(0, ggf)
                if i > 0:
                    ggb = sig(1, gpb)
                    xtb = tr(1, ggb)
                    dense([(0, i, xtf), (1, i - 1, xtb)])
                else:
                    dense([(0, i, xtf)])
            gpb = mm(1, UNROLL - 1)
            ggb = sig(1, gpb)
            xtb = tr(1, ggb)
            dense([(1, UNROLL - 1, xtb)])
            nc.sync.dma_start(y_out[0, :, :, ds(iv0, UNROLL), :], ys[0][:, :])
            nc.sync.dma_start(
                y_out[1, :, :, ds(t_steps - UNROLL - iv0, UNROLL), :],
                ys[1][:, :])

        tc.For_i_unrolled_general(0, t_steps, 1, body, max_unroll=UNROLL,
                                  hint_engines=(PE, DVE, ACT))


def _prep_host(seqs, lengths, weights, t_steps):
    """Permute gates [i,f,g,o]->[i,f,o,2g], transpose weights, mask offsets."""
    def perm(w):  # [4H, K] -> rows [i, f, o, 2g], transposed -> [K, 4H]
        return np.ascontiguousarray(
            np.concatenate([w[0:500], w[500:1000], w[1500:2000],
                            2.0 * w[1000:1500]], axis=0).T)

    def pb(b):
        return np.concatenate([b[0:500], b[500:1000], b[1500:2000],
                               2.0 * b[1000:1500]])[None, :]

    bf16 = mybir.dt.np(mybir.dt.bfloat16)
    nt = t_steps // 16
    wih0 = np.stack([perm(weights["W_ih0f"]), perm(weights["W_ih0b"])])
    wih1 = np.stack([perm(weights["W_ih1f"]), perm(weights["W_ih1b"])]).astype(bf16)
    whh = np.stack([
        np.stack([perm(weights["W_hh0f"]), perm(weights["W_hh0b"])]),
        np.stack([perm(weights["W_hh1f"]), perm(weights["W_hh1b"])]),
    ]).astype(bf16)
    bias = np.stack([
        np.stack([pb(weights["b0f"]), pb(weights["b0b"])]),
        np.stack([pb(weights["b1f"]), pb(weights["b1b"])]),
    ]).astype(bf16)
    xgid = np.zeros((9, 16), bf16)
    xgid[0:8, 0:8] = np.eye(8)
    xgid[0:8, 8:16] = np.eye(8)
    xgid[8, :] = 1.0
    idbf = np.eye(128, dtype=np.float32).astype(bf16)

    in_maps = []
    for c in range(NCORES):
        sl = slice(c * BS, (c + 1) * BS)
        m = (np.arange(t_steps)[None, :] < lengths[sl, None]).astype(np.float32)
        # moff[tt, 16t*8b, {-, +}]
        offc = MOFF * (1.0 - m)  # [8, T]
        mo = np.zeros((nt, 16, BS, 2), np.float32)
        mo[:, :, :, 0] = -offc.T.reshape(nt, 16, BS)
        mo[:, :, :, 1] = offc.T.reshape(nt, 16, BS)
        x = seqs[sl, :t_steps]  # [8, T, 512]
        x0T = np.ascontiguousarray(
            x.transpose(2, 1, 0).reshape(4, 128, t_steps * BS))
        in_maps.append({
            "x0T": x0T, "wih0": wih0, "wih1": wih1, "whh": whh, "bias": bias,
            "xgid": xgid, "idbf": idbf,
            "moff": mo.reshape(nt, 128, 2),
        })
    return in_maps


_CACHE = {}


def kernel(seqs, lengths, W_ih0f, W_hh0f, b0f, W_ih0b, W_hh0b, b0b,
           W_ih1f, W_hh1f, b1f, W_ih1b, W_hh1b, b1b, _collect=None):
    t_steps = TT
    seqs = np.asarray(seqs, np.float32)
    lengths = np.asarray(lengths)
    weights = dict(W_ih0f=W_ih0f, W_hh0f=W_hh0f, b0f=b0f, W_ih0b=W_ih0b,
                   W_hh0b=W_hh0b, b0b=b0b, W_ih1f=W_ih1f, W_hh1f=W_hh1f,
                   b1f=b1f, W_ih1b=W_ih1b, W_hh1b=W_hh1b, b1b=b1b)
    weights = {k: np.asarray(v, np.float32) for k, v in weights.items()}
    in_maps = _prep_host(seqs, lengths, weights, t_steps)

    if t_steps not in _CACHE:
        _CACHE[t_steps] = _build_nc(t_steps)
    nc = _CACHE[t_steps]

    res = run_bass_kernel_spmd(
        nc, in_maps, core_ids=list(range(NCORES)),
        trace=bool(os.environ.get("LSTM_TRACE")))
    if _collect is not None:
        _collect.append(res)
    # out is [2, 127, 4, T, 8] H-major per core -> [B, T, 2H]
    full = np.zeros((B, T, 2 * H), np.float32)
    for c in range(NCORES):
        r = np.asarray(res.results[c]["out"])
        for d in range(2):
            for k in range(4):
                off, cnt = RCH[k]
                full[c * BS:(c + 1) * BS, :t_steps, 500 * d + off:
                     500 * d + off + cnt] = r[d, :cnt, k].transpose(2, 1, 0)
    return full


if __name__ == "__main__":
    rng = np.random.default_rng(0)
    seqs = rng.standard_normal((B, T, D_IN), dtype=np.float32)
    lengths = rng.integers(1, T + 1, (B,))
    w = {}
    d_in = D_IN
    for l in range(2):
        for d in ("f", "b"):
            w[f"W_ih{l}{d}"] = (rng.standard_normal((G, d_in)) * 0.05).astype(np.float32)
            w[f"W_hh{l}{d}"] = (rng.standard_normal((G, H)) * 0.05).astype(np.float32)
            w[f"b{l}{d}"] = np.zeros(G, np.float32)
        d_in = 2 * H
    out = kernel(seqs, lengths, **w)
    print("out", out.shape, out.dtype, float(np.abs(out).max()))


# revision 6
# speedup vs baseline: 1.1878x; 1.1878x over previous
"""Trainium2 Bass kernel v2: 2-layer bidirectional AllenNLP LSTM.

B=64, T=512, D_IN=512, H=500. Data-parallel over batch: 8 seqs/core x 8 cores.

Design (per core):
- Input projection per layer: x_gates = x @ W_ih'.T; layer-0 stationary x^T
  comes pre-transposed from the host (x0T), layer-1 stationary comes directly
  from the H-major y0 intermediate, so no on-device transposes in inproj.
  Gate columns host-permuted to [i, f, o, 2*g]. All masking is folded into
  x_gates here: i/o gates get -50*(1-m), f gets +50*(1-m), so the recurrence
  needs zero mask ops and h == y exactly (validated vs reference).
- Recurrence: per direction, gates land in one PSUM tile [128, 500] with one
  gate per 32-partition column group (i@0, f@32, o@64, 2g@96): 16 matmuls
  (4 col groups x 4 K-chunks) with h^T stationary and W_hh streamed; the 4
  col groups run concurrently on the PE array. x_gates(t)+bias are injected
  via 8 identity rows + 1 ones row in K-chunk 0 (chunks 119+9/127/127/127).
- One sigmoid activation covers all gates (tanh(z) = 2*sig(2z)-1, g-gate
  pre-scaled by 2). The activated gate tile [128, 500] is PE-transposed in 4
  chunks into an H-major PSUM tile [127, 4x128]; the whole c/h elementwise
  chain then runs on dense [127, 4, 8] APs (~32 elem/lane) and the h op
  writes the next step's h^T stationary tile directly (bf16). No h transpose,
  no mask ops, no per-step DMA except the 2 x_gates injections.
- y output (== h) is staged H-major and DMA'd to an H-major DRAM layout;
  the host un-permutes. Layer-0 y feeds layer-1 inproj as its stationary.
"""

import os
import sys
from contextlib import ExitStack

import numpy as np

sys.path.insert(0, "/opt/trn_rl_repo")

import concourse.bass as bass
import concourse.bacc as bacc
import concourse.mybir as mybir
import concourse.tile as tile
from concourse.bass_utils import run_bass_kernel_spmd

B, T, D_IN, H, G = 64, 512, 512, 500, 2000  # G = 4*H
NCORES = 8
BS = B // NCORES  # 8 seqs per core
F32 = mybir.dt.float32
F32R = mybir.dt.float32r
BF16 = mybir.dt.bfloat16
ds = bass.ds
ts = bass.ts
PE = mybir.EngineType.PE
DVE = mybir.EngineType.DVE
ACT = mybir.EngineType.Activation
SIG = mybir.ActivationFunctionType.Sigmoid
TANH = mybir.ActivationFunctionType.Tanh
MUL = mybir.AluOpType.mult
ADD = mybir.AluOpType.add
SUB = mybir.AluOpType.subtract

# K chunks of the recurrence contraction: chunk 0 = h[0:119] + 8 identity
# rows (x_gates inject) + 1 ones row (bias inject) = 128.
RCH = [(0, 119), (119, 127), (246, 127), (373, 127)]
MOFF = 50.0  # mask offset magnitude on i/f/o pre-activations

UNROLL = 16
HB = 8  # steps per staging half-block

_env_t = os.environ.get("LSTM_T")
TT = int(_env_t) if _env_t else T  # reduced T for smoke tests


def _build_nc(t_steps: int):
    nt = t_steps // 16
    nc = bacc.Bacc("TRN2", target_bir_lowering=False, debug=False,
                   num_devices=NCORES)

    x0T = nc.dram_tensor("x0T", [4, 128, t_steps * BS], F32R,
                         kind="ExternalInput").ap()
    wih0 = nc.dram_tensor("wih0", [2, D_IN, G], F32R, kind="ExternalInput").ap()
    wih1 = nc.dram_tensor("wih1", [2, 2 * H, G], BF16, kind="ExternalInput").ap()
    whh = nc.dram_tensor("whh", [2, 2, H, G], BF16, kind="ExternalInput").ap()
    bias = nc.dram_tensor("bias", [2, 2, 1, G], BF16, kind="ExternalInput").ap()
    xgid = nc.dram_tensor("xgid", [9, 16], BF16, kind="ExternalInput").ap()
    idbf = nc.dram_tensor("idbf", [128, 128], BF16, kind="ExternalInput").ap()
    moff = nc.dram_tensor("moff", [nt, 128, 2], F32, kind="ExternalInput").ap()
    out = nc.dram_tensor("out", [2, 127, 4, t_steps, BS], F32,
                         kind="ExternalOutput").ap()

    xga = nc.dram_tensor("xga", [2, t_steps, BS, G], BF16, kind="Internal").ap()
    xgb = nc.dram_tensor("xgb", [2, t_steps, BS, G], BF16, kind="Internal").ap()
    y0h = nc.dram_tensor("y0h", [2, 127, 4, t_steps, BS], BF16,
                         kind="Internal").ap()

    with tile.TileContext(nc) as tc:
        with tc.tile_pool(name="gconst", bufs=1) as gconst:
            idt = gconst.tile([128, 128], BF16, tag="idt")
            nc.sync.dma_start(idt[:, :], idbf)
            _inproj0(nc, tc, x0T, wih0, moff, xga, nt)
            tc.strict_bb_all_engine_barrier()
            with ExitStack() as c0:
                st0 = _rec_prep(nc, tc, c0, 0, whh, bias, xgid)
                _rec(nc, tc, 0, t_steps, xga, st0, idt, y0h, BF16)
            tc.strict_bb_all_engine_barrier()
            _inproj1(nc, tc, y0h, wih1, moff, xgb, nt)
            tc.strict_bb_all_engine_barrier()
            with ExitStack() as c1:
                st1 = _rec_prep(nc, tc, c1, 1, whh, bias, xgid)
                _rec(nc, tc, 1, t_steps, xgb, st1, idt, out, F32)
    nc.compile()
    return nc


def _emit_gates(nc, pool, gp, mofft, xg_out, d, tt):
    """PSUM gate tile [128, 2048] -> bf16 with mask offsets -> DRAM."""
    for n in range(4):
        gs = pool.tile([128, 500], BF16, tag="gs")
        sl = gp[:, 512 * n:512 * n + 500]
        if n == 3:  # 2g: no offset
            nc.vector.tensor_copy(gs[:, :], sl)
        elif n == 1:  # f: +MOFF*(1-m)
            nc.vector.tensor_scalar(gs[:, :], sl, mofft[:, 1:2], None, ADD)
        else:  # i, o: -MOFF*(1-m)
            nc.vector.tensor_scalar(gs[:, :], sl, mofft[:, 0:1], None, ADD)
        nc.sync.dma_start(xg_out[d, ts(tt, 16), :, ts(n, 500)], gs[:, :])


def _inproj0(nc, tc, x0T, wih, moff, xg_out, nt):
    """xg_out[d, t, b, :] = x[t, b, :] @ wih[d] (+ mask offsets)."""
    with ExitStack() as ctx:
        wpool = ctx.enter_context(tc.tile_pool(name="ipw0", bufs=1))
        w_sb = []
        for d in range(2):
            row = []
            for k in range(4):
                t = wpool.tile([128, G], F32R, tag=f"w{d}_{k}")
                nc.sync.dma_start(t[:, :], wih[d, ts(k, 128), :])
                row.append(t)
            w_sb.append(row)
        pool = ctx.enter_context(tc.tile_pool(name="ip0", bufs=3))
        xpool = ctx.enter_context(tc.tile_pool(name="ipx0", bufs=2))
        psum = ctx.enter_context(
            tc.tile_pool(name="ipp0", bufs=2, space="PSUM"))

        for tt in range(nt):
            mofft = pool.tile([128, 2], F32, tag="moff")
            nc.gpsimd.dma_start(mofft[:, :], moff[tt])
            xT = []
            for k in range(4):
                xt = xpool.tile([128, 128], F32R, tag=f"xT{k}")
                nc.gpsimd.dma_start(xt[:, :], x0T[k, :, ts(tt, 128)])
                xT.append(xt)
            for d in range(2):
                gp = psum.tile([128, 2048], F32, tag="gp")
                for n in range(4):
                    for k in range(4):
                        nc.tensor.matmul(
                            gp[:, 512 * n:512 * n + 500], lhsT=xT[k][:, :],
                            rhs=w_sb[d][k][:, ts(n, 500)],
                            start=(k == 0), stop=(k == 3))
                _emit_gates(nc, pool, gp, mofft, xg_out, d, tt)


def _inproj1(nc, tc, y0h, wih, moff, xg_out, nt):
    """xg_out[d, t, b, :] = y0[t, b, :] @ wih[d]; stationary from H-major y0h."""
    with ExitStack() as ctx:
        wpool = ctx.enter_context(tc.tile_pool(name="ipw1", bufs=1))
        w_sb = []
        for d in range(2):
            row = []
            for dd in range(2):
                for k in range(4):
                    off, cnt = RCH[k]
                    t = wpool.tile([cnt, G], BF16, tag=f"w{d}_{dd}{k}")
                    nc.sync.dma_start(t[:, :],
                                      wih[d, ds(500 * dd + off, cnt), :])
                    row.append(t)
            w_sb.append(row)
        pool = ctx.enter_context(tc.tile_pool(name="ip1", bufs=3))
        xpool = ctx.enter_context(tc.tile_pool(name="ipx1", bufs=2))
        psum = ctx.enter_context(
            tc.tile_pool(name="ipp1", bufs=2, space="PSUM"))

        for tt in range(nt):
            mofft = pool.tile([128, 2], F32, tag="moff")
            nc.gpsimd.dma_start(mofft[:, :], moff[tt])
            xT = []
            for dd in range(2):
                for k in range(4):
                    off, cnt = RCH[k]
                    xt = xpool.tile([cnt, 128], BF16, tag=f"xT{dd}{k}")
                    nc.gpsimd.dma_start(xt[:, :], y0h[dd, 0:cnt, k, ts(tt, 16), :])
                    xT.append(xt)
            for d in range(2):
                gp = psum.tile([128, 2048], F32, tag="gp")
                for n in range(4):
                    for c in range(8):
                        cnt = RCH[c % 4][1]
                        nc.tensor.matmul(
                            gp[:, 512 * n:512 * n + 500],
                            lhsT=xT[c][0:cnt, :],
                            rhs=w_sb[d][c][0:cnt, ts(n, 500)],
                            start=(c == 0), stop=(c == 7))
                _emit_gates(nc, pool, gp, mofft, xg_out, d, tt)


def _rec_prep(nc, tc, ctx, layer, whh, bias, xgid):
    """Allocate recurrence tiles and issue W loads (overlaps prior phase)."""
    cpool = ctx.enter_context(tc.tile_pool(name=f"rc{layer}", bufs=1))
    # Streamed W tiles. Chunk 0 in a 4-deep rotation (rows 119:127 get
    # x_gates rows per step via static SBUF->SBUF DMA, row 127 = bias,
    # 3 steps of inject lead); chunks 1-3 static.
    rhs0, rhs_rest = [], []
    for d in range(2):
        quad = []
        for p in range(4):
            t = cpool.tile([128, G], BF16, tag=f"r0_{d}{p}")
            nc.sync.dma_start(t[0:119, :], whh[layer, d, 0:119, :])
            nc.sync.dma_start(t[127:128, :], bias[layer, d, :, :])
            quad.append(t)
        rhs0.append(quad)
        rest = []
        for k in range(1, 4):
            off, cnt = RCH[k]
            t = cpool.tile([cnt, G], BF16, tag=f"r{k}_{d}")
            nc.sync.dma_start(t[:, :], whh[layer, d, ds(off, cnt), :])
            rest.append(t)
        rhs_rest.append(rest)
    # Stationary h^T tiles per direction [128, 32]: col 8k+b; chunk-0
    # rows 119:128 hold the static identity8 + ones block. Separate
    # tiles per direction so the anti-phase pipeline has no false deps.
    sets = []
    for d in range(2):
        pair = []
        for p in range(2):
            t = cpool.tile([128, 32], BF16, tag=f"hT{d}{p}",
                           name=f"hT{d}{p}")
            nc.vector.memset(t[:, :], 0.0)
            nc.sync.dma_start(t[119:128, 0:8], xgid[0:9, 0:8])
            pair.append(t)
        sets.append(pair)
    # c state + dense scratch, all [127, 32] = (4 chunks x 8 batch)
    c_t = []
    for d in range(2):
        t = cpool.tile([127, 32], F32, tag=f"c{d}")
        nc.vector.memset(t[:, :], 0.0)
        c_t.append(t)
    stg = [cpool.tile([128, G], BF16, tag=f"stg{h}", name=f"stg{h}")
           for h in range(2)]
    # zero stationary for the PE-warming filler matmuls
    zt = cpool.tile([127, 8], BF16, tag="zt")
    nc.vector.memset(zt[:, :], 0.0)
    return rhs0, rhs_rest, sets, c_t, stg, zt


def _rec(nc, tc, layer, t_steps, xg, st, idt, y_out, y_dt):
    """Bidirectional recurrence; y == h streamed out H-major."""
    rhs0, rhs_rest, sets, c_t, stg, zt = st
    with ExitStack() as ctx:
        gpool = ctx.enter_context(
            tc.tile_pool(name=f"rg{layer}", bufs=2, space="PSUM"))
        tpool = ctx.enter_context(
            tc.tile_pool(name=f"rt{layer}", bufs=2, space="PSUM"))
        spool = ctx.enter_context(tc.tile_pool(name=f"rs{layer}", bufs=2))
        ypool = ctx.enter_context(tc.tile_pool(name=f"ry{layer}", bufs=2))

        tc.strict_bb_all_engine_barrier()

        def body(iv0, unroll):
            assert unroll == UNROLL
            for h in range(2):
                nc.gpsimd.dma_start(stg[h][0:64, :],
                                    xg[0, ds(iv0 + h * HB, HB), :, :])
                nc.gpsimd.dma_start(
                    stg[h][64:128, :],
                    xg[1, ds(t_steps - HB - iv0 - h * HB, HB), :, :])
            # y staging: [127, (k:4, i:16, b:8)] per direction, full block
            ys = [ypool.tile([127, 4 * UNROLL * BS], y_dt, tag=f"ys{d}",
                             name=f"ys{d}") for d in range(2)]
            def inject(d, j):
                half, jj = divmod(j, HB)
                row = 8 * jj if d == 0 else 64 + 8 * (HB - 1 - jj)
                eng = nc.sync if d == 0 else nc.gpsimd
                eng.dma_start(rhs0[d][j % 4][119:127, :],
                              stg[half][row:row + 8, :])

            def mm(d, j):
                gpd = gpool.tile([128, 500], F32, tag=f"gp{d}", name=f"gp{d}")
                # dep-free zero-stationary group first: adds 0 to PSUM but
                # streams during the wait-for-h window, keeping the PE busy
                # so HAM holds the warm clock
                for g in range(4):
                    nc.tensor.matmul(
                        gpd[32 * g:32 * g + 8, :],
                        lhsT=zt[0:127, 0:8],
                        rhs=rhs_rest[d][0][0:127, ts(g, 500)],
                        start=True, stop=False,
                        tile_position=(0, 32 * g))
                for k in (3, 1, 2, 0):
                    rt = rhs0[d][j % 4] if k == 0 else rhs_rest[d][k - 1]
                    kp = 128 if k == 0 else RCH[k][1]
                    for g in range(4):
                        nc.tensor.matmul(
                            gpd[32 * g:32 * g + 8, :],
                            lhsT=sets[d][j % 2][0:kp, 8 * k:8 * k + 8],
                            rhs=rt[0:kp, ts(g, 500)],
                            start=False, stop=(k == 0),
                            tile_position=(0, 32 * g))
                return gpd

            def sig(d, gpd):
                ggd = spool.tile([128, 500], BF16, tag=f"gg{d}", name=f"gg{d}")
                nc.scalar.activation(ggd[:, :], gpd[:, :], SIG)
                return ggd

            def tr(d, ggd):
                xt = tpool.tile([127, 512], BF16, tag=f"xt{d}", name=f"xt{d}")
                for k in range(4):
                    off, cnt = RCH[k]
                    nc.tensor.transpose(xt[0:cnt, ts(k, 128)],
                                        ggd[:, ds(off, cnt)],
                                        idt[0:128, 0:128])
                return xt

            def dense(specs):
                # specs: list of (d, j, xt); emit op-position-major so the
                # active chains interleave on the DVE FIFO
                st = []
                for d, j, xt in specs:
                    xr = xt[:, :].rearrange("p (k g b) -> p k g b", k=4, g=4)
                    gsc = spool.tile([127, 32], F32, tag=f"gsc{d}",
                                     name=f"gsc{d}")
                    igt = spool.tile([127, 32], F32, tag=f"ig{d}",
                                     name=f"ig{d}")
                    fct = spool.tile([127, 32], F32, tag=f"fc{d}",
                                     name=f"fc{d}")
                    csd = spool.tile([127, 32], F32, tag=f"cs{d}",
                                     name=f"cs{d}")
                    st.append((d, j, xr,
                               gsc[:, :].rearrange("p (k b) -> p k b", k=4),
                               igt[:, :].rearrange("p (k b) -> p k b", k=4),
                               fct[:, :].rearrange("p (k b) -> p k b", k=4),
                               c_t[d][:, :].rearrange("p (k b) -> p k b", k=4),
                               csd))
                for d, j, xr, gsr, igr, fcr, cr, csd in st:
                    nc.vector.tensor_scalar(gsr, xr[:, :, 3, 0:8],
                                            2.0, 1.0, MUL, SUB)  # g'=2s-1
                for d, j, xr, gsr, igr, fcr, cr, csd in st:
                    nc.vector.tensor_tensor(igr, xr[:, :, 0, 0:8], gsr, MUL)
                for d, j, xr, gsr, igr, fcr, cr, csd in st:
                    nc.vector.tensor_tensor(fcr, xr[:, :, 1, 0:8], cr, MUL)
                for d, j, xr, gsr, igr, fcr, cr, csd in st:
                    nc.vector.tensor_tensor(cr, igr, fcr, ADD)  # c_new
                for d, j, xr, gsr, igr, fcr, cr, csd in st:
                    nc.scalar.activation(csd[:, :], c_t[d][:, :], TANH)
                for d, j, xr, gsr, igr, fcr, cr, csd in st:
                    csr = csd[:, :].rearrange("p (k b) -> p k b", k=4)
                    # h = o * tanh(c), written straight into h^T stationary
                    sr = sets[d][1 - j % 2][:, :].rearrange(
                        "p (k b) -> p k b", k=4)
                    nc.vector.tensor_tensor(sr[0:119], xr[0:119, :, 2, 0:8],
                                            csr[0:119], MUL)
                    # rows 119:127 of chunks 1-3 (32-aligned base; rows
                    # 96:119 are recomputed with identical values)
                    nc.vector.tensor_tensor(sr[96:127, 1:4],
                                            xr[96:127, 1:4, 2, 0:8],
                                            csr[96:127, 1:4], MUL)
                    # y == h: stage H-major (bwd t-reversed within block)
                    yr = ys[d][:, :].rearrange("p (k i b) -> p k i b",
                                               k=4, i=UNROLL)
                    nc.gpsimd.tensor_copy(
                        yr[:, :, j if d == 0 else UNROLL - 1 - j, :],
                        sr[0:127])

            # anti-phase pipeline: bwd lags fwd by one step so each
            # direction's matmuls fill the other's elementwise window
            for i in range(unroll):
                inject(0, i)
                inject(1, i)
                gpf = mm(0, i)
                ggf = sig(0, gpf)
                if i > 0:
                    gpb = mm(1, i - 1)
                xtf = tr(0, ggf)
                if i > 0:
                    ggb = sig(1, gpb)
                    xtb = tr(1, ggb)
                    dense([(0, i, xtf), (1, i - 1, xtb)])
                else:
                    dense([(0, i, xtf)])
            gpb = mm(1, UNROLL - 1)
            ggb = sig(1, gpb)
            xtb = tr(1, ggb)
            dense([(1, UNROLL - 1, xtb)])
            nc.sync.dma_start(y_out[0, :, :, ds(iv0, UNROLL), :], ys[0][:, :])
            nc.sync.dma_start(
                y_out[1, :, :, ds(t_steps - UNROLL - iv0, UNROLL), :],
                ys[1][:, :])

        tc.For_i_unrolled_general(0, t_steps, 1, body, max_unroll=UNROLL,
                                  hint_engines=(PE, DVE, ACT))


def _prep_host(seqs, lengths, weights, t_steps):
    """Permute gates [i,f,g,o]->[i,f,o,2g], transpose weights, mask offsets."""
    def perm(w):  # [4H, K] -> rows [i, f, o, 2g], transposed -> [K, 4H]
        return np.ascontiguousarray(
            np.concatenate([w[0:500], w[500:1000], w[1500:2000],
                            2.0 * w[1000:1500]], axis=0).T)

    def pb(b):
        return np.concatenate([b[0:500], b[500:1000], b[1500:2000],
                               2.0 * b[1000:1500]])[None, :]

    bf16 = mybir.dt.np(mybir.dt.bfloat16)
    nt = t_steps // 16
    wih0 = np.stack([perm(weights["W_ih0f"]), perm(weights["W_ih0b"])])
    wih1 = np.stack([perm(weights["W_ih1f"]), perm(weights["W_ih1b"])]).astype(bf16)
    whh = np.stack([
        np.stack([perm(weights["W_hh0f"]), perm(weights["W_hh0b"])]),
        np.stack([perm(weights["W_hh1f"]), perm(weights["W_hh1b"])]),
    ]).astype(bf16)
    bias = np.stack([
        np.stack([pb(weights["b0f"]), pb(weights["b0b"])]),
        np.stack([pb(weights["b1f"]), pb(weights["b1b"])]),
    ]).astype(bf16)
    xgid = np.zeros((9, 16), bf16)
    xgid[0:8, 0:8] = np.eye(8)
    xgid[0:8, 8:16] = np.eye(8)
    xgid[8, :] = 1.0
    idbf = np.eye(128, dtype=np.float32).astype(bf16)

    in_maps = []
    for c in range(NCORES):
        sl = slice(c * BS, (c + 1) * BS)
        m = (np.arange(t_steps)[None, :] < lengths[sl, None]).astype(np.float32)
        # moff[tt, 16t*8b, {-, +}]
        offc = MOFF * (1.0 - m)  # [8, T]
        mo = np.zeros((nt, 16, BS, 2), np.float32)
        mo[:, :, :, 0] = -offc.T.reshape(nt, 16, BS)
        mo[:, :, :, 1] = offc.T.reshape(nt, 16, BS)
        x = seqs[sl, :t_steps]  # [8, T, 512]
        x0T = np.ascontiguousarray(
            x.transpose(2, 1, 0).reshape(4, 128, t_steps * BS))
        in_maps.append({
            "x0T": x0T, "wih0": wih0, "wih1": wih1, "whh": whh, "bias": bias,
            "xgid": xgid, "idbf": idbf,
            "moff": mo.reshape(nt, 128, 2),
        })
    return in_maps


_CACHE = {}


def kernel(seqs, lengths, W_ih0f, W_hh0f, b0f, W_ih0b, W_hh0b, b0b,
           W_ih1f, W_hh1f, b1f, W_ih1b, W_hh1b, b1b, _collect=None):
    t_steps = TT
    seqs = np.asarray(seqs, np.float32)
    lengths = np.asarray(lengths)
    weights = dict(W_ih0f=W_ih0f, W_hh0f=W_hh0f, b0f=b0f, W_ih0b=W_ih0b,
                   W_hh0b=W_hh0b, b0b=b0b, W_ih1f=W_ih1f, W_hh1f=W_hh1f,
                   b1f=b1f, W_ih1b=W_ih1b, W_hh1b=W_hh1b, b1b=b1b)
    weights = {k: np.asarray(v, np.float32) for k, v in weights.items()}
    in_maps = _prep_host(seqs, lengths, weights, t_steps)

    if t_steps not in _CACHE:
        _CACHE[t_steps] = _build_nc(t_steps)
    nc = _CACHE[t_steps]

    res = run_bass_kernel_spmd(
        nc, in_maps, core_ids=list(range(NCORES)),
        trace=bool(os.environ.get("LSTM_TRACE")))
    if _collect is not None:
        _collect.append(res)
    # out is [2, 127, 4, T, 8] H-major per core -> [B, T, 2H]
    full = np.zeros((B, T, 2 * H), np.float32)
    for c in range(NCORES):
        r = np.asarray(res.results[c]["out"])
        for d in range(2):
            for k in range(4):
                off, cnt = RCH[k]
                full[c * BS:(c + 1) * BS, :t_steps, 500 * d + off:
                     500 * d + off + cnt] = r[d, :cnt, k].transpose(2, 1, 0)
    return full


if __name__ == "__main__":
    rng = np.random.default_rng(0)
    seqs = rng.standard_normal((B, T, D_IN), dtype=np.float32)
    lengths = rng.integers(1, T + 1, (B,))
    w = {}
    d_in = D_IN
    for l in range(2):
        for d in ("f", "b"):
            w[f"W_ih{l}{d}"] = (rng.standard_normal((G, d_in)) * 0.05).astype(np.float32)
            w[f"W_hh{l}{d}"] = (rng.standard_normal((G, H)) * 0.05).astype(np.float32)
            w[f"b{l}{d}"] = np.zeros(G, np.float32)
        d_in = 2 * H
    out = kernel(seqs, lengths, **w)
    print("out", out.shape, out.dtype, float(np.abs(out).max()))
